# revision 1
# baseline (speedup 1.0000x reference)
"""Causal attention (B=4, S=2048, D=1024, fp32 in/out) on 8 Trainium2 cores.

Sharding: core c = (batch b = c//2, variant h = c%2). Each core computes the
attention output for 1024 of the 2048 query rows of one batch element.

Load balancing ("sorted-slot" assignment): variant A owns global q-tiles
(0,1,2,3,12,13,14,15), variant B owns (4..11).  Slot i on every core
processes keys [0, CNT[i]*128) with CNT = (5,6,7,8,13,14,15,16), which
dominates both variants' causal needs, so a single NEFF (identical loop
structure) serves all 8 cores; per-core differences are carried entirely by
input data (pre-sliced/pre-transposed X, packed additive mask).

K/V are not recomputed per core: core (b, h) projects K^T/V only for its
own key half [h*1024, (h+1)*1024), and the pair exchanges halves with an
AllGather over replica groups [[0,1],[2,3],[4,5],[6,7]] through DRAM
bounce buffers (collectives can't touch I/O tensors).

Matmul operands are bf16 (fp32 matmul runs at half rate on the PE and
disables FWL); accumulation stays fp32 in PSUM and the softmax runs in
fp32, so the result error stays at the ~1e-3 level.

Per-core kernel:
  Phase A: Q^T = (Wq/32)^T Xq^T ; K^T_loc = Wk^T X_loc^T ; V_loc = X_loc Wv;
           AllGather K^T/V halves into SBUF-resident kT / v_sb (bf16).
  Phase B: per q-tile slot: S = Q^T.T K^T (PSUM fp32, accum over e), +mask
           on eviction (DVE, fp32), exp -> bf16 with fused fp32 row-sum
           (ScalarE accum_out), PE transpose of P tiles, O = P^T.T V
           accumulated in PSUM over key tiles, normalized by 1/rowsum on
           eviction (ScalarE Copy with scale=AP).

No max-subtraction in softmax: logits are (q.k)/32 with std ~0.33, bounded
by ~+-2, so exp() is perfectly conditioned; masked entries use -1e4.
"""

import numpy as np
from contextlib import ExitStack

import ml_dtypes

import concourse.bass as bass
import concourse.tile as tile
from concourse import bacc, mybir
from concourse.bass_utils import run_bass_kernel_spmd

P = 128
B, S, D = 4, 2048, 1024
NCORES = 8
DT = D // P      # 8 contraction tiles
ST = S // P      # 16 key tiles (global)
SLOC = S // 2    # 1024 local keys per core
SLT = SLOC // P  # 8 local key tiles
ET = D // P      # 8 output-feature tiles
QLOC = 1024      # query rows per core
QT = QLOC // P   # 8 local q tiles

G_A = (0, 1, 2, 3, 12, 13, 14, 15)   # variant A global q-tiles (slot order)
G_B = (4, 5, 6, 7, 8, 9, 10, 11)     # variant B
CNT = (5, 6, 7, 8, 13, 14, 15, 16)   # key tiles per slot (shared structure)
# Scores are computed transposed (S^T[k, q], keys on partitions).  Because
# CNT is ascending, the slots active for key-tile kt form a contiguous
# q-suffix starting at slot JKT[kt]; WKT[kt] is that suffix's width.
JKT = tuple(next(i for i in range(QT) if CNT[i] > kt) for kt in range(ST))
WKT = tuple((QT - j) * P for j in JKT)
OFFKT = tuple(int(x) for x in np.cumsum((0,) + WKT)[:-1])
MASK_COLS = sum(WKT)                 # 10752
NEG = -10000.0

F32 = mybir.dt.float32
BF16 = mybir.dt.bfloat16

REPLICA_GROUPS = [[0, 1], [2, 3], [4, 5], [6, 7]]


def _chunks(width, step=512):
    out = []
    c0 = 0
    while c0 < width:
        out.append((c0, min(step, width - c0)))
        c0 += out[-1][1]
    return out


def _build(reps=1):
    nc = bacc.Bacc("TRN2", target_bir_lowering=False, debug=False,
                   num_devices=NCORES)
    xt_in = nc.dram_tensor("xt", [D, SLOC], BF16, kind="ExternalInput").ap()
    xqt_in = nc.dram_tensor("xqt", [D, QLOC], BF16, kind="ExternalInput").ap()
    wq_in = nc.dram_tensor("wq", [D, D], BF16, kind="ExternalInput").ap()
    wk_in = nc.dram_tensor("wk", [D, D], BF16, kind="ExternalInput").ap()
    wv_in = nc.dram_tensor("wv", [D, D], BF16, kind="ExternalInput").ap()
    mask_in = nc.dram_tensor("mask", [P, MASK_COLS], BF16,
                             kind="ExternalInput").ap()
    out = nc.dram_tensor("out", [QLOC, D], F32, kind="ExternalOutput").ap()

    with tile.TileContext(nc) as tc, ExitStack() as ctx:
        persist = ctx.enter_context(tc.tile_pool(name="persist", bufs=1))
        kT = persist.tile([P, ET, S], BF16, tag="kT")      # K^T [e%128, et, key]
        qT = persist.tile([P, ET, QLOC], BF16, tag="qT")   # Q^T [e%128, et, q]
        v_sb = persist.tile([P, ST, D], BF16, tag="v")     # V   [k%128, kt, e]
        ones = persist.tile([P, 1], BF16, tag="ones")
        nc.gpsimd.memset(ones[:], 1.0)

        for _rep in range(reps):
            _emit_body(nc, tc, _rep, xt_in, xqt_in, wq_in, wk_in, wv_in,
                       mask_in, out, kT, qT, v_sb, ones)
    nc.compile()
    return nc


def _emit_body(nc, tc, rep, xt_in, xqt_in, wq_in, wk_in, wv_in, mask_in, out,
               kT, qT, v_sb, ones):
    body = ExitStack()
    # Masks are pure inputs; prefetch them from phase A so score evictions
    # (DVE add) never stall on mask arrival and back up PSUM.
    mpool = body.enter_context(tc.tile_pool(name="m", bufs=6))
    masks = []

    def _load_mask(kt):
        w = WKT[kt]
        m_t = mpool.tile([P, 8 * P], BF16, tag="m", name="m_t")[:, :w]
        nc.scalar.dma_start(m_t, mask_in[:, OFFKT[kt]:OFFKT[kt] + w])
        masks.append(m_t)

    def _prefetch_masks():
        # Only as many as the pool holds without waiting -- a waiting DMA
        # would head-of-line-block the ACT queue (kbounce/vbounce follow).
        for kt in range(6):
            _load_mask(kt)

    # ---------------- Phase A : projections + KV exchange ----------------
    with ExitStack() as pa:
        xp = pa.enter_context(tc.tile_pool(name="xp", bufs=1))
        dp = pa.enter_context(tc.tile_pool(name="dp", bufs=1, space="DRAM"))
        psA = pa.enter_context(tc.tile_pool(name="psA", bufs=8, space="PSUM"))

        # Critical-path inputs (K-proj needs wk+xt) on the SP queue, split
        # per d-tile so the first matmul starts as soon as slice 0 lands;
        # the rest stream on the ACT queue in parallel.
        xt = xp.tile([P, DT, SLOC], BF16, tag="xt")
        wq_t = xp.tile([P, DT, D], BF16, tag="wq")
        wk_t = xp.tile([P, DT, D], BF16, tag="wk")
        wv_t = xp.tile([P, DT, D], BF16, tag="wv")
        xqt = xp.tile([P, DT, QLOC], BF16, tag="xqt")
        for dt in range(DT):
            nc.sync.dma_start(wk_t[:, dt, :], wk_in[dt * P:(dt + 1) * P, :])
            nc.sync.dma_start(xt[:, dt, :], xt_in[dt * P:(dt + 1) * P, :])
        for dt in range(DT):
            nc.scalar.dma_start(wv_t[:, dt, :], wv_in[dt * P:(dt + 1) * P, :])
        for dt in range(DT):
            nc.scalar.dma_start(xqt[:, dt, :], xqt_in[dt * P:(dt + 1) * P, :])
        for dt in range(DT):
            nc.scalar.dma_start(wq_t[:, dt, :], wq_in[dt * P:(dt + 1) * P, :])
        _prefetch_masks()

        # Tiny warm-up collective: absorbs the cc firmware's first-use setup
        # latency while the input DMAs stream, so the real K gather is fast.
        warm_in = dp.tile([P, 8], BF16, tag="warm_in")
        warm_out = dp.tile([2 * P, 8], BF16, tag="warm_out")
        nc.gpsimd.collective_compute(
            "AllGather", mybir.AluOpType.bypass,
            replica_groups=REPLICA_GROUPS,
            ins=[warm_in.opt()], outs=[warm_out.opt()])

        klocal = xp.tile([P, ET, SLOC], BF16, tag="klocal")
        vlocal = xp.tile([P, SLT, D], BF16, tag="vlocal")
        kbounce = dp.tile([D, SLOC], BF16, tag="kbounce")
        kgather = dp.tile([2 * D, SLOC], BF16, tag="kgather")
        vbounce = dp.tile([SLOC, D], BF16, tag="vbounce")
        vgather = dp.tile([2 * SLOC, D], BF16, tag="vgather")

        # K^T_loc[et, k] = sum_d Wk[d, et].T X_loc^T[d, k]
        # dt is the OUTER loop (8 PSUM groups per half) so matmuls start as
        # soon as the first wk/xt slices land instead of waiting for all 8.
        for half in range(2):
            groups = [(et, kc) for et in range(half * 4, half * 4 + 4)
                      for kc in range(2)]
            pss = [psA.tile([P, 512], F32, tag="ps", name="ps")
                   for _ in groups]
            for dt in range(DT):
                for gi, (et, kc) in enumerate(groups):
                    nc.tensor.matmul(
                        pss[gi][:], lhsT=wk_t[:, dt, et * P:(et + 1) * P],
                        rhs=xt[:, dt, kc * 512:(kc + 1) * 512],
                        start=(dt == 0), stop=(dt == DT - 1))
            for gi, (et, kc) in enumerate(groups):
                nc.vector.tensor_copy(
                    klocal[:, et, kc * 512:(kc + 1) * 512], pss[gi][:])
        nc.scalar.dma_start(
            kbounce.rearrange("(et p) k -> p et k", p=P), klocal[:])
        nc.gpsimd.collective_compute(
            "AllGather", mybir.AluOpType.bypass,
            replica_groups=REPLICA_GROUPS,
            ins=[kbounce.opt()], outs=[kgather.opt()])
        # The SP queue is FIFO, so the bounce-out stores must not sit
        # behind these gather-dependent loads: stores go on the ACT queue.
        for et in range(ET):
            for r in range(2):
                nc.sync.dma_start(
                    kT[:, et, r * SLOC:(r + 1) * SLOC],
                    kgather[r * D + et * P: r * D + (et + 1) * P, :])

        # V_loc[kt, e] = sum_d X_loc^T[d, kt].T Wv[d, e]
        for half in range(2):
            groups = [(st, ec) for st in range(half * 4, half * 4 + 4)
                      for ec in range(2)]
            pss = [psA.tile([P, 512], F32, tag="ps", name="ps")
                   for _ in groups]
            for dt in range(DT):
                for gi, (st, ec) in enumerate(groups):
                    nc.tensor.matmul(
                        pss[gi][:], lhsT=xt[:, dt, st * P:(st + 1) * P],
                        rhs=wv_t[:, dt, ec * 512:(ec + 1) * 512],
                        start=(dt == 0), stop=(dt == DT - 1))
            for gi, (st, ec) in enumerate(groups):
                nc.vector.tensor_copy(
                    vlocal[:, st, ec * 512:(ec + 1) * 512], pss[gi][:])
        nc.scalar.dma_start(
            vbounce.rearrange("(st p) e -> p st e", p=P), vlocal[:])
        nc.gpsimd.collective_compute(
            "AllGather", mybir.AluOpType.bypass,
            replica_groups=REPLICA_GROUPS,
            ins=[vbounce.opt()], outs=[vgather.opt()])
        for kt in range(ST):
            nc.sync.dma_start(v_sb[:, kt, :],
                              vgather[kt * P:(kt + 1) * P, :])

        # Q^T[et, q] = sum_d Wq[d, et].T Xq^T[d, q]
        for half in range(2):
            groups = [(et, qc) for et in range(half * 4, half * 4 + 4)
                      for qc in range(2)]
            pss = [psA.tile([P, 512], F32, tag="ps", name="ps")
                   for _ in groups]
            for dt in range(DT):
                for gi, (et, qc) in enumerate(groups):
                    nc.tensor.matmul(
                        pss[gi][:], lhsT=wq_t[:, dt, et * P:(et + 1) * P],
                        rhs=xqt[:, dt, qc * 512:(qc + 1) * 512],
                        start=(dt == 0), stop=(dt == DT - 1))
            for gi, (et, qc) in enumerate(groups):
                nc.vector.tensor_copy(
                    qT[:, et, qc * 512:(qc + 1) * 512], pss[gi][:])

    # ---------------- Phase B : attention (transposed scores) ----------
    # S^T[k, q] with keys on partitions: slot layouts make the active slots
    # for key-tile kt a contiguous q-suffix, so one PSUM strip per kt.
    # exp(S^T) directly yields P^T -- the AV stationary operand -- with no
    # PE transposes; row-sums come from a ones-vector matmul fused into
    # the AV weight loads.
    with body, ExitStack() as pb:
        stile = pb.enter_context(tc.tile_pool(name="st", bufs=1))
        sT = stile.tile([P, ST, QLOC], F32, tag="sT")   # S^T [k%128, kt, q]
        # per-slot P^T tiles so an early slot's AV only waits its own exp
        ptpool = pb.enter_context(tc.tile_pool(name="pt", bufs=QT))
        opool = pb.enter_context(tc.tile_pool(name="o", bufs=2))
        stpool = pb.enter_context(tc.tile_pool(name="stat", bufs=QT))
        psS = pb.enter_context(tc.tile_pool(name="psS", bufs=2, space="PSUM"))
        psAV = pb.enter_context(tc.tile_pool(name="psAV", bufs=3, space="PSUM"))
        psRS = pb.enter_context(tc.tile_pool(name="psRS", bufs=1, space="PSUM"))

        pTs = {}
        for kt in range(ST):
            jq = JKT[kt] * P
            w = WKT[kt]
            if kt >= 6:
                _load_mask(kt)
            ps = psS.tile([P, 8 * P], F32, tag="psS", name="ps")[:, :w]
            for et in range(ET):
                for c0, cw in _chunks(w):
                    nc.tensor.matmul(
                        ps[:, c0:c0 + cw],
                        lhsT=kT[:, et, kt * P:(kt + 1) * P],
                        rhs=qT[:, et, jq + c0:jq + c0 + cw],
                        start=(et == 0), stop=(et == ET - 1))
            nc.vector.tensor_tensor(
                sT[:, kt, jq:QLOC], ps[:, :w], masks[kt][:, :w],
                op=mybir.AluOpType.add)
            # fire exp for every slot whose last key-tile this was
            for i in range(QT):
                if CNT[i] == kt + 1:
                    pT_i = ptpool.tile([P, ST, P], BF16, tag="pt", name="pT_i")
                    nc.scalar.activation(
                        pT_i[:, 0:CNT[i], :],
                        sT[:, 0:CNT[i], i * P:(i + 1) * P],
                        mybir.ActivationFunctionType.Exp)
                    pTs[i] = pT_i

        for i in range(QT):
            ck = CNT[i]
            pT_i = pTs[i]
            psavs = [psAV.tile([P, 512], F32, tag="psAV", name="psavs")
                     for _ in range(2)]
            psrs = psRS.tile([P, 1], F32, tag="psRS", name="psrs")
            for kt in range(ck):
                lhsT = pT_i[:, kt, :]
                nc.tensor.matmul(psrs[:], lhsT=lhsT, rhs=ones[:],
                                 start=(kt == 0), stop=(kt == ck - 1))
                for ec in range(2):
                    nc.tensor.matmul(
                        psavs[ec][:], lhsT=lhsT,
                        rhs=v_sb[:, kt, ec * 512:(ec + 1) * 512],
                        start=(kt == 0), stop=(kt == ck - 1))

            recip = stpool.tile([P, 1], F32, tag="rc", name="recip")
            nc.vector.reciprocal(recip[:], psrs[:])
            for ec in range(2):
                o_t = opool.tile([P, 512], F32, tag="o", name="o_t")
                nc.scalar.activation(o_t[:], psavs[ec][:],
                                     mybir.ActivationFunctionType.Copy,
                                     scale=recip[:])
                nc.sync.dma_start(
                    out[i * P:(i + 1) * P, ec * 512:(ec + 1) * 512], o_t[:])


_COMPILED = None


def _get_compiled():
    global _COMPILED
    if _COMPILED is None:
        _COMPILED = _build()
    return _COMPILED


def _qrows(G):
    return np.concatenate([np.arange(g * P, (g + 1) * P) for g in G])


def _host_mask(G):
    # Transposed additive mask: for key-tile kt the active slots are the
    # q-suffix JKT[kt]..7; column c = (slot_index - JKT[kt])*128 + q_in_tile,
    # row r = key_in_tile.  0 where key <= query position, NEG otherwise.
    m = np.full((P, MASK_COLS), NEG, np.float32)
    for kt in range(ST):
        key = kt * P + np.arange(P)[:, None]
        qpos = np.concatenate(
            [G[i] * P + np.arange(P) for i in range(JKT[kt], QT)])[None, :]
        m[:, OFFKT[kt]:OFFKT[kt] + WKT[kt]] = np.where(
            key <= qpos, np.float32(0.0), np.float32(NEG))
    return m.astype(ml_dtypes.bfloat16)


def _host_in_maps(X, Wq, Wk, Wv):
    bf = ml_dtypes.bfloat16
    X = np.asarray(X, np.float32)
    wq_s = (np.asarray(Wq, np.float32) / np.float32(np.sqrt(D))).astype(bf)
    wk = np.asarray(Wk, np.float32).astype(bf)
    wv = np.asarray(Wv, np.float32).astype(bf)
    masks = {0: _host_mask(G_A), 1: _host_mask(G_B)}
    qr = {0: _qrows(G_A), 1: _qrows(G_B)}
    in_maps = []
    for c in range(NCORES):
        b, h = divmod(c, 2)
        Xb = X[b]
        in_maps.append({
            "xt": np.ascontiguousarray(Xb[h * SLOC:(h + 1) * SLOC].T).astype(bf),
            "xqt": np.ascontiguousarray(Xb[qr[h]].T).astype(bf),
            "wq": wq_s, "wk": wk, "wv": wv,
            "mask": masks[h],
        })
    return in_maps, qr


def kernel(X, Wq, Wk, Wv, _trace=False):
    nc = _get_compiled()
    in_maps, qr = _host_in_maps(X, Wq, Wk, Wv)
    res = run_bass_kernel_spmd(nc, in_maps, core_ids=list(range(NCORES)),
                               trace=_trace)
    O = np.empty((B, S, D), np.float32)
    for c in range(NCORES):
        b, h = divmod(c, 2)
        O[b, qr[h]] = res.results[c]["out"]
    if _trace:
        kernel._last_exec_time_ns = res.exec_time_ns
        kernel._last_results = res
    return O



# revision 2
# speedup vs baseline: 1.0692x; 1.0692x over previous
"""Causal attention (B=4, S=2048, D=1024, fp32 in/out) on 8 Trainium2 cores.

Sharding: core c = (batch b = c//2, variant h = c%2). Queries are split at
64-row granularity: global 64-row chunk g (g=0..31 per batch) goes to
variant g%2, laid out in ascending order, so core column x maps to global
query row 128*(x//64) + 64*h + x%64.

This interleave makes the kernel's causal structure variant-INDEPENDENT:
  * scores strip for key tile kt covers exactly columns [64*kt, 1024) --
    68 (128x128x1024) tile-equivalents per core, the tile-granular minimum.
  * AV "slot" t = columns [128*t, 128*t+128) needs key tiles 0..2t+1
    (CNT = 2t+2 for every core) -- 72 tile-equivalents.
  * the causal mask reduces to ONE kt-independent [128, 64] tile applied to
    the first 64 columns of every strip (the diagonal chunk).
(The previous revision used 128-row q-tiles with a sorted-slot assignment,
costing 84+84 tile-equivalents and a 2.7MB precomputed mask.)

K/V are not recomputed per core: core (b, h) projects K^T/V only for its
own key half, and the pair exchanges halves with an AllGather over replica
groups [[0,1],[2,3],[4,5],[6,7]] through DRAM bounce buffers.

Phase order is K proj -> V proj -> Q proj so both gather round-trips hide
under projection matmuls, and ~24 dummy matmuls at t=0 warm the PE HAM
clock gate (cold PE runs at 1.2GHz for the first ~3.4us of activity) while
the input DMAs stream in on the sync (wk,wq) and act (xt,wv,xqt) queues.

Phase B per key tile kt: S^T strip [128 keys x (16-kt)*64 q] accumulated
over 8 e-tiles in PSUM; DVE adds the diagonal mask in-place in PSUM;
ScalarE exps straight from PSUM into per-slot bf16 P^T tiles (no SBUF
staging of fp32 scores). After strip 2t+1, slot t's AV runs: O = P^T.T V
accumulated over its 2t+2 key tiles with a fused ones-matmul row-sum,
normalized by 1/rowsum on eviction (ScalarE Copy with scale=AP).

Matmul operands are bf16 (fp32 matmul runs at half rate and disables FWL);
accumulation stays fp32 in PSUM; softmax exp in fp32 from PSUM. No
max-subtraction: logits are (q.k)/32, std ~0.33, and masked entries get
-1e4 -> exp underflows to 0.
"""

import numpy as np
from contextlib import ExitStack

import ml_dtypes

import concourse.bass as bass
import concourse.tile as tile
from concourse import bacc, mybir
from concourse.bass_utils import run_bass_kernel_spmd

P = 128
B, S, D = 4, 2048, 1024
NCORES = 8
DT = D // P      # 8 contraction tiles
ST = S // P      # 16 key tiles (global)
SLOC = S // 2    # 1024 local keys per core
ET = D // P      # 8 output-feature tiles
QLOC = 1024      # query rows per core
NSLOT = 8        # AV slots of 128 query columns
CNT = tuple(2 * t + 2 for t in range(NSLOT))   # key tiles per slot
NEG = -10000.0
NWARM = 24       # HAM warm-up matmuls

F32 = mybir.dt.float32
BF16 = mybir.dt.bfloat16

REPLICA_GROUPS = [[0, 1], [2, 3], [4, 5], [6, 7]]


def _chunks(width, step=512):
    out = []
    c0 = 0
    while c0 < width:
        out.append((c0, min(step, width - c0)))
        c0 += out[-1][1]
    return out


def _build():
    nc = bacc.Bacc("TRN2", target_bir_lowering=False, debug=False,
                   num_devices=NCORES)
    xt_in = nc.dram_tensor("xt", [D, SLOC], BF16, kind="ExternalInput").ap()
    xqt_in = nc.dram_tensor("xqt", [D, QLOC], BF16, kind="ExternalInput").ap()
    wq_in = nc.dram_tensor("wq", [D, D], BF16, kind="ExternalInput").ap()
    wk_in = nc.dram_tensor("wk", [D, D], BF16, kind="ExternalInput").ap()
    wv_in = nc.dram_tensor("wv", [D, D], BF16, kind="ExternalInput").ap()
    mask_in = nc.dram_tensor("mask", [P, 64], BF16, kind="ExternalInput").ap()
    out = nc.dram_tensor("out", [QLOC, D], F32, kind="ExternalOutput").ap()

    with tile.TileContext(nc) as tc, ExitStack() as ctx:
        persist = ctx.enter_context(tc.tile_pool(name="persist", bufs=1))
        kT = persist.tile([P, ET, S], BF16, tag="kT")      # K^T [e%128, et, key]
        qT = persist.tile([P, ET, QLOC], BF16, tag="qT")   # Q^T [e%128, et, q]
        v_sb = persist.tile([P, ST, D], BF16, tag="v")     # V   [k%128, kt, e]
        ones = persist.tile([P, 1], BF16, tag="ones")
        mask = persist.tile([P, 64], BF16, tag="mask")
        warm = persist.tile([P, 512], BF16, tag="warm")
        nc.gpsimd.memset(ones[:], 1.0)
        nc.gpsimd.memset(warm[:], 0.25)

        _emit_body(nc, tc, xt_in, xqt_in, wq_in, wk_in, wv_in, mask_in, out,
                   kT, qT, v_sb, ones, mask, warm)
    nc.compile()
    return nc


def _emit_body(nc, tc, xt_in, xqt_in, wq_in, wk_in, wv_in, mask_in, out,
               kT, qT, v_sb, ones, mask, warm):
    # ---------------- Phase A : projections + KV exchange ----------------
    with ExitStack() as pa:
        xp = pa.enter_context(tc.tile_pool(name="xp", bufs=1))
        dp = pa.enter_context(tc.tile_pool(name="dp", bufs=1, space="DRAM"))
        psA = pa.enter_context(tc.tile_pool(name="psA", bufs=8, space="PSUM"))

        # PE warm-up: keeps the HAM activity window busy from t~1us so the
        # first real matmul (waiting on wk/xt DMA) already runs at 2.4GHz.
        psw = psA.tile([P, 512], F32, tag="ps", name="psw")
        for _ in range(NWARM):
            nc.tensor.matmul(psw[:], lhsT=warm[:, 0:P], rhs=warm[:],
                             start=True, stop=True)

        xt = xp.tile([P, DT, SLOC], BF16, tag="xt")
        wq_t = xp.tile([P, DT, D], BF16, tag="wq")
        wk_t = xp.tile([P, DT, D], BF16, tag="wk")
        wv_t = xp.tile([P, DT, D], BF16, tag="wv")
        xqt = xp.tile([P, DT, QLOC], BF16, tag="xqt")
        # K proj needs wk+xt slices in dt order: stream them on separate
        # queues in parallel.  wq/xqt/wv follow behind on the same queues.
        for dt in range(DT):
            nc.sync.dma_start(wk_t[:, dt, :], wk_in[dt * P:(dt + 1) * P, :])
            nc.scalar.dma_start(xt[:, dt, :], xt_in[dt * P:(dt + 1) * P, :])
        for dt in range(DT):
            nc.sync.dma_start(wq_t[:, dt, :], wq_in[dt * P:(dt + 1) * P, :])
            nc.scalar.dma_start(wv_t[:, dt, :], wv_in[dt * P:(dt + 1) * P, :])
        for dt in range(DT):
            nc.scalar.dma_start(xqt[:, dt, :], xqt_in[dt * P:(dt + 1) * P, :])
        nc.gpsimd.dma_start(mask[:], mask_in[:, :])

        # Tiny warm-up collective: absorbs the cc firmware's first-use setup
        # latency while the input DMAs stream, so the real K gather is fast.
        warm_in = dp.tile([P, 8], BF16, tag="warm_in")
        warm_out = dp.tile([2 * P, 8], BF16, tag="warm_out")
        nc.gpsimd.collective_compute(
            "AllGather", mybir.AluOpType.bypass,
            replica_groups=REPLICA_GROUPS,
            ins=[warm_in.opt()], outs=[warm_out.opt()])

        klocal = xp.tile([P, ET, SLOC], BF16, tag="klocal")
        vlocal = xp.tile([P, ST // 2, D], BF16, tag="vlocal")
        kbounce = dp.tile([D, SLOC], BF16, tag="kbounce")
        kgather = dp.tile([2 * D, SLOC], BF16, tag="kgather")
        vbounce = dp.tile([SLOC, D], BF16, tag="vbounce")
        vgather = dp.tile([2 * SLOC, D], BF16, tag="vgather")

        # K^T_loc[et, k] = sum_d Wk[d, et].T X_loc^T[d, k].  dt is the OUTER
        # loop so matmuls start as soon as the first wk/xt slices land.
        for half in range(2):
            groups = [(et, kc) for et in range(half * 4, half * 4 + 4)
                      for kc in range(2)]
            pss = [psA.tile([P, 512], F32, tag="ps", name="ps")
                   for _ in groups]
            for dt in range(DT):
                for gi, (et, kc) in enumerate(groups):
                    nc.tensor.matmul(
                        pss[gi][:], lhsT=wk_t[:, dt, et * P:(et + 1) * P],
                        rhs=xt[:, dt, kc * 512:(kc + 1) * 512],
                        start=(dt == 0), stop=(dt == DT - 1))
            for gi, (et, kc) in enumerate(groups):
                nc.vector.tensor_copy(
                    klocal[:, et, kc * 512:(kc + 1) * 512], pss[gi][:])
        nc.scalar.dma_start(
            kbounce.rearrange("(et p) k -> p et k", p=P), klocal[:])
        nc.gpsimd.collective_compute(
            "AllGather", mybir.AluOpType.bypass,
            replica_groups=REPLICA_GROUPS,
            ins=[kbounce.opt()], outs=[kgather.opt()])
        # Gather output is replica-rank ordered = global key order on both
        # cores of a pair, so these loads are variant-independent.
        for r in range(2):
            for et in range(ET):
                nc.sync.dma_start(
                    kT[:, et, r * SLOC:(r + 1) * SLOC],
                    kgather[r * D + et * P: r * D + (et + 1) * P, :])

        # V_loc[kt, e] = sum_d X_loc^T[d, kt].T Wv[d, e]
        for half in range(2):
            groups = [(st, ec) for st in range(half * 4, half * 4 + 4)
                      for ec in range(2)]
            pss = [psA.tile([P, 512], F32, tag="ps", name="ps")
                   for _ in groups]
            for dt in range(DT):
                for gi, (st, ec) in enumerate(groups):
                    nc.tensor.matmul(
                        pss[gi][:], lhsT=xt[:, dt, st * P:(st + 1) * P],
                        rhs=wv_t[:, dt, ec * 512:(ec + 1) * 512],
                        start=(dt == 0), stop=(dt == DT - 1))
            for gi, (st, ec) in enumerate(groups):
                nc.vector.tensor_copy(
                    vlocal[:, st, ec * 512:(ec + 1) * 512], pss[gi][:])
        nc.scalar.dma_start(
            vbounce.rearrange("(st p) e -> p st e", p=P), vlocal[:])
        nc.gpsimd.collective_compute(
            "AllGather", mybir.AluOpType.bypass,
            replica_groups=REPLICA_GROUPS,
            ins=[vbounce.opt()], outs=[vgather.opt()])
        # Tile-ascending so early AV slots' tiles land first.
        for kt in range(ST):
            nc.sync.dma_start(v_sb[:, kt, :],
                              vgather[kt * P:(kt + 1) * P, :])

        # Q^T[et, q] = sum_d Wq[d, et].T Xq^T[d, q]
        for half in range(2):
            groups = [(et, qc) for et in range(half * 4, half * 4 + 4)
                      for qc in range(2)]
            pss = [psA.tile([P, 512], F32, tag="ps", name="ps")
                   for _ in groups]
            for dt in range(DT):
                for gi, (et, qc) in enumerate(groups):
                    nc.tensor.matmul(
                        pss[gi][:], lhsT=wq_t[:, dt, et * P:(et + 1) * P],
                        rhs=xqt[:, dt, qc * 512:(qc + 1) * 512],
                        start=(dt == 0), stop=(dt == DT - 1))
            for gi, (et, qc) in enumerate(groups):
                nc.vector.tensor_copy(
                    qT[:, et, qc * 512:(qc + 1) * 512], pss[gi][:])

    # ---------------- Phase B : attention (transposed scores) ----------
    with ExitStack() as pb:
        ptpool = pb.enter_context(tc.tile_pool(name="pt", bufs=1))
        opool = pb.enter_context(tc.tile_pool(name="o", bufs=2))
        stpool = pb.enter_context(tc.tile_pool(name="stat", bufs=4))
        psS = pb.enter_context(tc.tile_pool(name="psS", bufs=2, space="PSUM"))
        psAV = pb.enter_context(tc.tile_pool(name="psAV", bufs=2, space="PSUM"))
        psRS = pb.enter_context(tc.tile_pool(name="psRS", bufs=2, space="PSUM"))

        # Per-slot P^T tiles [k%128, kt, q].  Column range [0,64) of key
        # tile 2t+1 is causally dead (never written by exp) -> zero it once.
        pTs = []
        for t in range(NSLOT):
            pT_t = ptpool.tile([P, CNT[t], P], BF16, tag=f"pt{t}")
            nc.vector.memset(pT_t[:, CNT[t] - 1, 0:64], 0.0)
            pTs.append(pT_t)

        def _emit_av(t):
            ck = CNT[t]
            pT_t = pTs[t]
            psavs = [psAV.tile([P, 512], F32, tag="psAV", name="psavs")
                     for _ in range(2)]
            psrs = psRS.tile([P, 1], F32, tag="psRS", name="psrs")
            for kt in range(ck):
                lhsT = pT_t[:, kt, :]
                for ec in range(2):
                    nc.tensor.matmul(
                        psavs[ec][:], lhsT=lhsT,
                        rhs=v_sb[:, kt, ec * 512:(ec + 1) * 512],
                        start=(kt == 0), stop=(kt == ck - 1))
                nc.tensor.matmul(psrs[:], lhsT=lhsT, rhs=ones[:],
                                 start=(kt == 0), stop=(kt == ck - 1))
            recip = stpool.tile([P, 1], F32, tag="rc", name="recip")
            nc.vector.reciprocal(recip[:], psrs[:])
            for ec in range(2):
                o_t = opool.tile([P, 512], F32, tag="o", name="o_t")
                nc.scalar.activation(o_t[:], psavs[ec][:],
                                     mybir.ActivationFunctionType.Copy,
                                     scale=recip[:])
                nc.sync.dma_start(
                    out[t * P:(t + 1) * P, ec * 512:(ec + 1) * 512], o_t[:])

        for kt in range(ST):
            w = 64 * (ST - kt)              # strip covers columns [64kt, 1024)
            ps = psS.tile([P, 8 * P], F32, tag="psS", name="ps")[:, :w]
            for c0, cw in _chunks(w):
                for et in range(ET):
                    nc.tensor.matmul(
                        ps[:, c0:c0 + cw],
                        lhsT=kT[:, et, kt * P:(kt + 1) * P],
                        rhs=qT[:, et, 64 * kt + c0:64 * kt + c0 + cw],
                        start=(et == 0), stop=(et == ET - 1))
            # Diagonal chunk (first 64 cols): causal mask, in-place in PSUM.
            nc.vector.tensor_tensor(ps[:, 0:64], ps[:, 0:64], mask[:],
                                    op=mybir.AluOpType.add)
            # exp straight from PSUM into each overlapping slot's P^T tile.
            for t in range(kt // 2, NSLOT):
                lo = max(128 * t, 64 * kt)
                hi = 128 * t + 128
                sl = lo - 64 * kt
                tl = lo - 128 * t
                nc.scalar.activation(
                    pTs[t][:, kt, tl:tl + (hi - lo)], ps[:, sl:sl + (hi - lo)],
                    mybir.ActivationFunctionType.Exp)
            if kt % 2 == 1:
                _emit_av((kt - 1) // 2)


_COMPILED = None


def _get_compiled():
    global _COMPILED
    if _COMPILED is None:
        _COMPILED = _build()
    return _COMPILED


def _qrows(h):
    # core column x -> global query row 128*(x//64) + 64*h + x%64
    return np.concatenate(
        [np.arange(128 * p + 64 * h, 128 * p + 64 * h + 64)
         for p in range(QLOC // 64)])


def _host_mask(h):
    # Diagonal-chunk mask, identical for every key tile kt: key r (within
    # tile) vs column j of the chunk at global row 128kt + 64h + j.
    r = np.arange(P)[:, None]
    j = np.arange(64)[None, :]
    m = np.where(r > j + 64 * h, np.float32(NEG), np.float32(0.0))
    return m.astype(ml_dtypes.bfloat16)


def _host_in_maps(X, Wq, Wk, Wv):
    bf = ml_dtypes.bfloat16
    X = np.asarray(X, np.float32)
    wq_s = (np.asarray(Wq, np.float32) / np.float32(np.sqrt(D))).astype(bf)
    wk = np.asarray(Wk, np.float32).astype(bf)
    wv = np.asarray(Wv, np.float32).astype(bf)
    masks = {0: _host_mask(0), 1: _host_mask(1)}
    qr = {0: _qrows(0), 1: _qrows(1)}
    in_maps = []
    for c in range(NCORES):
        b, h = divmod(c, 2)
        Xb = X[b]
        in_maps.append({
            "xt": np.ascontiguousarray(Xb[h * SLOC:(h + 1) * SLOC].T).astype(bf),
            "xqt": np.ascontiguousarray(Xb[qr[h]].T).astype(bf),
            "wq": wq_s, "wk": wk, "wv": wv,
            "mask": masks[h],
        })
    return in_maps, qr


def kernel(X, Wq, Wk, Wv, _trace=False):
    nc = _get_compiled()
    in_maps, qr = _host_in_maps(X, Wq, Wk, Wv)
    res = run_bass_kernel_spmd(nc, in_maps, core_ids=list(range(NCORES)),
                               trace=_trace)
    O = np.empty((B, S, D), np.float32)
    for c in range(NCORES):
        b, h = divmod(c, 2)
        O[b, qr[h]] = res.results[c]["out"]
    if _trace:
        kernel._last_exec_time_ns = res.exec_time_ns
        kernel._last_results = res
    return O
